# revision 24
# baseline (speedup 1.0000x reference)
"""MLA decode (DeepSeek-V3 dims, absorbed attention) on 8 Trainium2 NeuronCores.

Sharding (v6):
  - wq_a/wkv_a output-sharded (192/72 dims per core) -> AllGather [32,264]
  - wq_b / wkv_b head-sharded (16 heads per core); q_abs absorbed on producer
  - AllToAll #1 redistributes q (+ fresh kv/pe token) to batch-sharded layout
  - attention batch-sharded (4 batches per core); caches host-pretransposed to
    [c, t]; V tiles rebuilt via PE transposes; softmax without max-subtraction
  - AllToAll #2 (split in two batch-halves for overlap) back to head-sharded
    for the uv projection -> o_uv^T tiles
  - wo column-sharded by local heads -> partial [32, 7168] finished with a
    two-part ReduceScatter (split by output columns for overlap)
v6 changes vs v5:
  - wqkva and wkv_b_nope pretiled on host -> contiguous line-rate DMAs
  - wq_b fully prefetched during phase 1 / AllGather window
  - PE warm-up chain at t=0 + keep-warm poke matmuls tied to prefetch DMAs
    (fights HAM re-throttle during collective waits)
  - col-tiled (tile_position) matmuls for the M=32 GEMMs (phase 2 and the
    wo projection): 3-4 subtiles run concurrently on the PE array
  - wo tiles grown to [128, 2048] (512 KB DMAs), issued on the Scalar HWDGE
    queue so they never block the Sync queue that feeds attention
  - AllToAll #2 and ReduceScatter split in halves, overlapped with compute
All heavy streams are fp16; accumulations, norms and softmax stats are fp32.
"""
from contextlib import ExitStack

import numpy as np

import concourse.bacc as bacc
import concourse.tile as tile
import concourse.mybir as mybir
from concourse.bass_utils import run_bass_kernel_spmd
from concourse import masks

F32 = mybir.dt.float32
F16 = mybir.dt.float16
AF = mybir.ActivationFunctionType

DIM = 7168; H = 128; QLR = 1536; KVLR = 512
DN = 128; DR = 64; DV = 128; QKD = DN + DR
B = 32; MAXS = 4096; SPOS = 4095
SCALE = QKD ** -0.5
EPS = 1e-6

NCORES = 8
HL = H // NCORES          # 16 local heads
BL = B // NCORES          # 4 local batches
QL = QLR // NCORES        # 192 wq_a out dims per core
KL = (KVLR + DR) // NCORES  # 72 wkv_a out dims per core
PL = QL + KL              # 264 phase-1 out dims per core
CKV = KVLR + DR           # 576
NKT = DIM // 128          # 56 k-tiles of phase 1
NT = MAXS // 128          # 32 t-tiles
TB = 8                    # t-tiles per block
NTB = NT // TB            # 4 blocks per batch
KD = HL * DV              # 2048 contraction dims of wo per core
NK5 = KD // 128           # 16 wo k-tiles
# phase-5 wo tiling: quad widths (sum = DIM); tiles [128, W5[q]]
W5 = [4096, 2048, 1024]
W5OFF = [0, 4096, 6144]
NQUAD = len(W5)
# prefetched wo tiles (consumption order ti = q*NK5 + k):
WO_A, WO_A2, WO_C, WO_B1, WO_B2 = 4, 2, 4, 2, 2

_CACHE = {}

# bisection flags
F_POKE = int(__import__('os').environ.get('F_POKE', '1'))
F_COLT2 = int(__import__('os').environ.get('F_COLT2', '1'))
F_COLT5 = int(__import__('os').environ.get('F_COLT5', '1'))
F_SPLIT = int(__import__('os').environ.get('F_SPLIT', '1'))
F_DBG = int(__import__('os').environ.get('F_DBG', '0'))
F_A2A2V5 = int(__import__('os').environ.get('F_A2A2V5', '1'))
F_SPLIT2 = int(__import__('os').environ.get('F_SPLIT2', '1'))


def _build(spmd=True):
    nc = bacc.Bacc("TRN2", target_bir_lowering=False, debug=False,
                   enable_asserts=False, num_devices=NCORES if spmd else 1)

    def collective(kind, op, replica_groups, ins, outs):
        if spmd:
            nc.gpsimd.collective_compute(kind, op, replica_groups=replica_groups,
                                         ins=ins, outs=outs)
            return
        i_ap, o_ap = ins[0], outs[0]
        ni = i_ap.shape[0]
        if kind == "AllGather":
            for r in range(o_ap.shape[0] // ni):
                nc.sync.dma_start(o_ap[r * ni:(r + 1) * ni], i_ap)
        elif kind == "ReduceScatter":
            nc.sync.dma_start(o_ap, i_ap[:o_ap.shape[0]])
        else:
            nc.sync.dma_start(o_ap, i_ap)
    din = {}
    def inp(name, shape, dt=F16):
        din[name] = nc.dram_tensor(name, list(shape), dt, kind="ExternalInput").ap()
        return din[name]

    x_p = inp("x_p", [128, NKT * B])                 # x^T pre-tiled to SBUF layout
    wqkva_p = inp("wqkva_p", [128, NKT * PL])        # [wq_a^T|wkv_a^T] SBUF layout
    wq_b_t = inp("wq_b_t", [QLR, HL * QKD])          # wq_b^T col slice (my heads)
    wn_p = inp("wn_p", [128, HL * KVLR])             # wkv_b nope rows, SBUF layout
    wuv_p = inp("wuv_p", [128, HL * (KVLR // 128) * DV])  # uv weights, SBUF layout
    wo_th = inp("wo_th", [KD, DIM])                  # wo^T rows for my heads
    kv_t = inp("kv_t", [BL, KVLR, MAXS])             # latent cache^T, my batches
    pe_t = inp("pe_t", [BL, DR, MAXS])               # pe cache^T, my batches
    kv_norm_w = inp("kv_norm_w", [B, KVLR], F32)
    fcos = inp("fcos", [B, DR // 2], F32)
    fsin = inp("fsin", [B, DR // 2], F32)

    out_part = nc.dram_tensor("out_part", [BL, DIM], F16, kind="ExternalOutput").ap()
    if F_DBG:
        dbg_qlr = nc.dram_tensor("dbg_qlr", [B, QLR], F32, kind="ExternalOutput").ap()
        dbg_qsb = nc.dram_tensor("dbg_qsb", [B, HL * QKD], F32, kind="ExternalOutput").ap()
        dbg_obT = nc.dram_tensor("dbg_obT", [128, HL * B], F16, kind="ExternalOutput").ap()

    RG = [list(range(NCORES))]

    with tile.TileContext(nc) as tc:
        with ExitStack() as _es:
            cpool = _es.enter_context(tc.tile_pool(name="const", bufs=1))
            dram = _es.enter_context(tc.tile_pool(name="dram", bufs=1, space="DRAM"))
            p5wa = _es.enter_context(tc.tile_pool(name="p5wa", bufs=WO_A))
            p4w = _es.enter_context(tc.tile_pool(name="p4w", bufs=1))
            p5keep = _es.enter_context(tc.tile_pool(name="p5keep", bufs=1))
            flags = _es.enter_context(tc.tile_pool(name="flags", bufs=2))
            warmps = _es.enter_context(tc.tile_pool(name="warmps", bufs=1, space="PSUM"))
            ident = cpool.tile([128, 128], F32)
            masks.make_identity(nc, ident[:])
            identh_t = cpool.tile([128, 128], F16)
            nc.vector.tensor_copy(identh_t[:], ident[:])
            identh = identh_t[:]

            # PE warm-up chain: back-to-back matmuls on the identity while the
            # first DMAs stream, so HAM un-throttles before phase-1 math
            wps = warmps.tile([128, 512], F32)
            if F_POKE:
                for wi in range(50):
                    nc.tensor.matmul(wps[:, :128], identh[:, :], identh[:, :],
                                     start=True, stop=True)

            def poke(src_ap):
                if F_POKE:
                    nc.tensor.matmul(wps[:1, :1], src_ap, src_ap, start=True, stop=True)

            def warm_burst(dep_ap, n=36):
                # ~3.4us of back-to-back matmuls gated on dep_ap's producer:
                # forces the HAM activity monitor back to full clock before a
                # compute phase starts (PE transposes alone don't count)
                if F_POKE:
                    nc.tensor.matmul(wps[:1, :1], dep_ap, dep_ap, start=True, stop=True)
                    for _ in range(n):
                        nc.tensor.matmul(wps[:, :128], identh[:, :], identh[:, :],
                                         start=True, stop=True)

            # persistent collective DRAM buffers
            agw_in = dram.tile([1, 8], F32)
            agw_out = dram.tile([NCORES, 8], F32, addr_space="Shared" if spmd else "Local")
            ag1_in = dram.tile([B, PL], F32)
            ag1_out = dram.tile([NCORES, B, PL], F32, addr_space="Shared" if spmd else "Local")
            a2a1_in = dram.tile([NCORES, BL, HL + 1, CKV], F16)
            a2a1_out = dram.tile([NCORES, BL, HL + 1, CKV], F16)
            if F_SPLIT2:
                a2a2_in_a = dram.tile([NCORES, 2, HL, KVLR], F16)
                a2a2_out_a = dram.tile([NCORES, 2, HL, KVLR], F16)
                a2a2_in_b = dram.tile([NCORES, 2, HL, KVLR], F16)
                a2a2_out_b = dram.tile([NCORES, 2, HL, KVLR], F16)
            else:
                a2a2_in = dram.tile([NCORES, BL, HL, KVLR], F16)
                a2a2_out = dram.tile([NCORES, BL, HL, KVLR], F16)
            rs_in_a = dram.tile([B, 4096], F16)
            rs_out_a = dram.tile([BL, 4096], F16)
            rs_in_b = dram.tile([B, 2048], F16)
            rs_out_b = dram.tile([BL, 2048], F16)
            rs_in_c = dram.tile([B, 1024], F16)
            rs_out_c = dram.tile([BL, 1024], F16)

            # o_uv^T tiles, produced by phase 4, consumed by phase 5
            obT_all = p5keep.tile([128, HL * B], F16)

            # ---- wo prefetch tiles; all on the Scalar HWDGE queue so they
            # never head-of-line-block the Sync queue (kv/attention stream).
            # Consumption order ti = q*NK5 + k (quad-major).
            wo_tiles = {}

            # warm-up collective: absorbs the cc-stream init barrier (~40-50us)
            # concurrently with phase 1 instead of serializing before the
            # first real AllGather
            warm_sb = cpool.tile([1, 8], F32)
            nc.gpsimd.memset(warm_sb[:], 0.0)
            nc.gpsimd.dma_start(agw_in[:], warm_sb[:])
            if spmd:
                collective("AllGather", mybir.AluOpType.bypass, RG,
                           [agw_in[:].flatten()], [agw_out[:].flatten()])

            def wo_tile_dma(ti, pool):
                q, k = divmod(ti, NK5)
                wt = pool.tile([128, W5[q]], F16, tag=f"p5w{W5[q]}")
                wo_tiles[ti] = wt
                nc.scalar.dma_start(
                    wt[:], wo_th[k * 128:(k + 1) * 128,
                                 W5OFF[q]:W5OFF[q] + W5[q]])
                return wt

            def wo_gate_release(tis, pool, src_ap, do_poke=False):
                # gpsimd writes one element of each tile (WAW) right after
                # src_ap's producer, then the DMA may start
                for ti in tis:
                    q, k = divmod(ti, NK5)
                    wt = pool.tile([128, W5[q]], F16, tag=f"p5w{W5[q]}")
                    wo_tiles[ti] = wt
                    nc.gpsimd.tensor_copy(wt[:1, :1], src_ap)
                    nc.scalar.dma_start(
                        wt[:], wo_th[k * 128:(k + 1) * 128,
                                     W5OFF[q]:W5OFF[q] + W5[q]])
                    if do_poke:
                        poke(wt[:32, :1])

            NG0 = WO_A
            NG1 = NG0 + WO_A2
            NGC = [NG1, NG1 + 2, NG1 + 3, NG1 + WO_C]  # C released in 3 steps
            NG2 = NG1 + WO_C + WO_B1
            NG3 = NG2 + WO_B2

            # ================= Phases 1-2 scope =================
            with ExitStack() as _es12:
                p2wq = _es12.enter_context(tc.tile_pool(name="p2wq", bufs=12))
                pb = _es12.enter_context(tc.tile_pool(name="p1b", bufs=1))
                p2wn = _es12.enter_context(tc.tile_pool(name="p2wn", bufs=1))

                # ---------------- Phase 1: q_lr & kv_full partials ----------------
                KCH = 14                       # k-tiles per wqkva chunk
                with tc.tile_pool(name="p1sb", bufs=1) as p1sb, \
                     tc.tile_pool(name="p1w", bufs=4) as p1w, \
                     tc.tile_pool(name="p1ps", bufs=1, space="PSUM") as p1ps:
                    xT = p1sb.tile([128, NKT * B], F16)
                    nc.sync.dma_start(xT[:], x_p[:])
                    wch = []
                    for c4 in range(NKT // KCH):
                        wt = p1w.tile([128, KCH * PL], F16, tag="p1w")
                        nc.sync.dma_start(
                            wt[:], wqkva_p[:, c4 * KCH * PL:(c4 + 1) * KCH * PL])
                        wch.append(wt)
                    # wq_b prefetch: stream all 12 row-blocks during phase1+AG
                    wqb_tiles = []
                    for k in range(QLR // 128):
                        wt = p2wq.tile([128, HL * QKD], F16, tag="p2wq")
                        nc.sync.dma_start(wt[:], wq_b_t[k * 128:(k + 1) * 128, :])
                        wqb_tiles.append(wt)
                    ps1 = p1ps.tile([B, PL], F32)
                    for c4 in range(NKT // KCH):
                        for j in range(KCH):
                            k = c4 * KCH + j
                            nc.tensor.matmul(ps1[:], xT[:, k * B:(k + 1) * B],
                                             wch[c4][:, j * PL:(j + 1) * PL],
                                             start=(k == 0), stop=(k == NKT - 1))
                    st1 = p1sb.tile([B, PL], F32)
                    nc.vector.tensor_copy(st1[:], ps1[:])
                    nc.gpsimd.dma_start(ag1_in[:], st1[:])
                    collective("AllGather", mybir.AluOpType.bypass, RG,
                               [ag1_in[:].flatten()], [ag1_out[:].flatten()])


                # wn / wuv / wo-A gated on the last wq_b tile so they never
                # compete with it for HBM bandwidth
                wn_all = p2wn.tile([128, HL * KVLR], F16, tag="wn")
                nc.gpsimd.tensor_copy(wn_all[:1, :1], wqb_tiles[-1][:1, :1])
                nc.scalar.dma_start(wn_all[:], wn_p[:])
                wuv = p4w.tile([128, HL * (KVLR // 128) * DV], F16, tag="wuv")
                nc.gpsimd.tensor_copy(wuv[:1, :1], wqb_tiles[-1][:1, :1])
                nc.scalar.dma_start(wuv[:], wuv_p[:])
                wo_gate_release(range(NG0), p5wa, wqb_tiles[-1][:1, :1])

                # ---------------- Phase 1b: rmsnorm + rope (all batches) ----------------
                q_lr = pb.tile([B, QLR], F32)
                # gather [32, 8, 192] -> [32, 1536]
                nc.gpsimd.dma_start(
                    q_lr[:].rearrange("b (r q) -> b r q", r=NCORES),
                    ag1_out[:, :, :QL].rearrange("r b q -> b r q"))
                kvf = pb.tile([B, CKV], F32)
                nc.gpsimd.dma_start(
                    kvf[:].rearrange("b (r q) -> b r q", r=NCORES),
                    ag1_out[:, :, QL:].rearrange("r b q -> b r q"))
                warm_burst(q_lr[:32, :1])

                # rmsnorm(q_lr): elementwise weight folded into wq_b on host;
                # the 1/rms scale is applied to phase-2 matmul outputs.
                eps_t = pb.tile([B, 1], F32)
                nc.gpsimd.memset(eps_t[:], EPS)
                qs = pb.tile([B, 1], F32)
                sq_tmp = pb.tile([B, QLR], F32)
                nc.scalar.activation(sq_tmp[:], q_lr[:], AF.Square, accum_out=qs[:])
                nc.scalar.activation(qs[:], qs[:], AF.Sqrt, scale=1.0 / QLR, bias=eps_t[:])
                nc.vector.reciprocal(qs[:], qs[:])
                if F_DBG:
                    nc.gpsimd.dma_start(dbg_qlr, q_lr[:])

                # q_lr^T tiles for phase 2
                qlrT = pb.tile([128, (QLR // 128) * B], F16)
                with tc.tile_pool(name="p1bps", bufs=2, space="PSUM") as pbps:
                    for k in range(QLR // 128):
                        pT = pbps.tile([128, B], F32, tag="p1bT")
                        nc.tensor.transpose(pT[:], q_lr[:, k * 128:(k + 1) * 128], ident[:B, :B])
                        nc.vector.tensor_copy(qlrT[:, k * B:(k + 1) * B], pT[:])

                # kvpe_new = [rmsnorm(kv) | rope(k_pe)]
                kvpe_new = pb.tile([B, CKV], F32)
                ks = pb.tile([B, 1], F32)
                kv_tmp = pb.tile([B, KVLR], F32)
                nc.scalar.activation(kv_tmp[:], kvf[:, :KVLR], AF.Square, accum_out=ks[:])
                nc.scalar.activation(ks[:], ks[:], AF.Sqrt, scale=1.0 / KVLR, bias=eps_t[:])
                nc.vector.reciprocal(ks[:], ks[:])
                nc.vector.tensor_copy(kvpe_new[:, :KVLR], kvf[:, :KVLR])
                nc.vector.tensor_scalar_mul(kvpe_new[:, :KVLR], kvpe_new[:, :KVLR], ks[:])
                knw = pb.tile([B, KVLR], F32)
                nc.sync.dma_start(knw[:], kv_norm_w[:])
                nc.vector.tensor_tensor(kvpe_new[:, :KVLR], kvpe_new[:, :KVLR],
                                        knw[:], mybir.AluOpType.mult)

                cosb = pb.tile([B, DR // 2], F32)
                sinb = pb.tile([B, DR // 2], F32)
                nc.sync.dma_start(cosb[:], fcos[:])
                nc.sync.dma_start(sinb[:], fsin[:])

                pe_src = kvf[:, KVLR:].rearrange("b (i two) -> b i two", two=2)
                pe_dst = kvpe_new[:, KVLR:].rearrange("b (i two) -> b i two", two=2)
                t1r = pb.tile([B, DR // 2], F32)
                t2r = pb.tile([B, DR // 2], F32)
                x1, x2 = pe_src[:, :, 0], pe_src[:, :, 1]
                nc.vector.tensor_tensor(t1r[:], x1, cosb[:], mybir.AluOpType.mult)
                nc.vector.tensor_tensor(t2r[:], x2, sinb[:], mybir.AluOpType.mult)
                nc.vector.tensor_tensor(pe_dst[:, :, 0], t1r[:], t2r[:], mybir.AluOpType.subtract)
                nc.vector.tensor_tensor(t1r[:], x1, sinb[:], mybir.AluOpType.mult)
                nc.vector.tensor_tensor(t2r[:], x2, cosb[:], mybir.AluOpType.mult)
                nc.vector.tensor_tensor(pe_dst[:, :, 1], t1r[:], t2r[:], mybir.AluOpType.add)

                # send fresh kv/pe rows through A2A slot h==HL (fp16 payload)
                kvpe_h = pb.tile([B, CKV], F16)
                nc.vector.tensor_copy(kvpe_h[:], kvpe_new[:])
                nc.gpsimd.dma_start(a2a1_in[:, :, HL, :], kvpe_h[:])

                # ---------------- Phase 2: q = q_lr_n @ wq_b^T, rope, absorb ----------------
                # col-tiled: 3 n-blocks of 512 run concurrently per k-tile
                with tc.tile_pool(name="p2", bufs=1) as p2, \
                     tc.tile_pool(name="p2st", bufs=3) as p2st:
                    NQ = HL * QKD  # 3072
                    HG = HL // 2   # heads per group
                    NQG = HG * QKD  # 1536 columns per group
                    q_sb = p2.tile([B, NQ], F32)
                    rope_q = p2.tile([B, HL, DR], F32)
                    rope_q2 = rope_q[:].rearrange("b h (i two) -> b h i two", two=2)
                    cb = cosb[:].rearrange("b (h i) -> b h i", h=1).to_broadcast((B, HG, DR // 2))
                    sb_ = sinb[:].rearrange("b (h i) -> b h i", h=1).to_broadcast((B, HG, DR // 2))
                    t1 = p2.tile([B, HL * DR // 2], F32)
                    t2 = p2.tile([B, HL * DR // 2], F32)

                    with tc.tile_pool(name="p2ps", bufs=2, space="PSUM") as p2ps, \
                         tc.tile_pool(name="p2ps2", bufs=2, space="PSUM") as p2ps2:
                      flag_p2 = flags.tile([1, 1], F16, tag="flag_p2")
                      # 1) both groups' q matmuls first (keeps TensorE dense)
                      psqs = []
                      for g in range(2):
                        gc0 = g * NQG
                        psq = p2ps.tile([128, 512], F32, tag="psq", name=f"psq{g}")
                        psqs.append(psq)
                        for k in range(QLR // 128):
                            for n in range(NQG // 512):
                                nc.tensor.matmul(
                                    psq[32 * n:32 * n + B, :],
                                    qlrT[:, k * B:(k + 1) * B],
                                    wqb_tiles[k][:, gc0 + n * 512:gc0 + (n + 1) * 512],
                                    start=(k == 0), stop=(k == QLR // 128 - 1),
                                    tile_position=(0, 32 * n))
                        for n in range(NQG // 512):
                            nc.vector.tensor_scalar_mul(
                                q_sb[:, gc0 + n * 512:gc0 + (n + 1) * 512],
                                psq[32 * n:32 * n + B, :], qs[:])

                      # 2) rope all heads in one pass
                      cba = cosb[:].rearrange("b (h i) -> b h i", h=1).to_broadcast((B, HL, DR // 2))
                      sba = sinb[:].rearrange("b (h i) -> b h i", h=1).to_broadcast((B, HL, DR // 2))
                      qpe2a = q_sb[:].rearrange("b (h d) -> b h d", h=HL)[:, :, DN:] \
                          .rearrange("b h (i two) -> b h i two", two=2)
                      t1v = t1[:].rearrange("b (h i) -> b h i", h=HL)
                      t2v = t2[:].rearrange("b (h i) -> b h i", h=HL)
                      x1, x2 = qpe2a[:, :, :, 0], qpe2a[:, :, :, 1]
                      nc.vector.tensor_tensor(t1v, x1, cba, mybir.AluOpType.mult)
                      nc.vector.tensor_tensor(t2v, x2, sba, mybir.AluOpType.mult)
                      nc.vector.tensor_tensor(rope_q2[:, :, :, :, 0] if False else rope_q2[:, :, :, 0], t1v, t2v, mybir.AluOpType.subtract)
                      nc.vector.tensor_tensor(t1v, x1, sba, mybir.AluOpType.mult)
                      nc.vector.tensor_tensor(t2v, x2, cba, mybir.AluOpType.mult)
                      nc.vector.tensor_tensor(rope_q2[:, :, :, 1], t1v, t2v, mybir.AluOpType.add)

                      # 3) absorb quads of heads; alternate copy engines
                      for hp_g in range(HL // 4):
                            stage = p2st.tile([B, 4 * CKV], F16, tag="stage")
                            for j in range(4):
                                h = 4 * hp_g + j
                                qnT = p2ps2.tile([DN, B], F32, tag="qnT")
                                nc.tensor.transpose(
                                    qnT[:], q_sb[:, h * QKD:h * QKD + DN],
                                    ident[:B, :B])
                                qnTs = p2st.tile([DN, B], F16, tag="qnTs")
                                pabs = p2ps2.tile([B, KVLR], F32, tag="pabs")
                                if j % 2 == 0:
                                    nc.vector.tensor_copy(qnTs[:], qnT[:])
                                else:
                                    nc.scalar.copy(qnTs[:], qnT[:])
                                nc.tensor.matmul(pabs[:], qnTs[:],
                                                 wn_all[:, h * KVLR:(h + 1) * KVLR],
                                                 start=True, stop=True)
                                if j % 2 == 0:
                                    nc.vector.tensor_copy(
                                        stage[:, j * CKV:j * CKV + KVLR], pabs[:])
                                    nc.vector.tensor_copy(
                                        stage[:, j * CKV + KVLR:(j + 1) * CKV],
                                        rope_q[:, h, :])
                                else:
                                    nc.scalar.copy(
                                        stage[:, j * CKV:j * CKV + KVLR], pabs[:])
                                    nc.scalar.copy(
                                        stage[:, j * CKV + KVLR:(j + 1) * CKV],
                                        rope_q[:, h, :])
                            nc.gpsimd.dma_start(
                                a2a1_in[:, :, 4 * hp_g:4 * hp_g + 4, :],
                                stage[:].rearrange("b (j c) -> b j c", j=4))
                            if hp_g == HL // 4 - 1:
                                # flag: A2 wo prefetch releases off this point
                                nc.vector.tensor_copy(flag_p2[:], stage[:1, :1])

                    if F_DBG:
                        nc.gpsimd.dma_start(dbg_qsb, q_sb[:])
                    collective("AllToAll", mybir.AluOpType.bypass, RG,
                               [a2a1_in[:].flatten()], [a2a1_out[:].flatten()])

            # ---------------- Phase 3: attention, batch-sharded ----------------
            _es35 = _es.enter_context(ExitStack())
            p5wa2 = _es35.enter_context(tc.tile_pool(name="p5wa2", bufs=WO_A2))
            p5wc = _es35.enter_context(tc.tile_pool(name="p5wc", bufs=WO_C))
            p5wb1 = _es35.enter_context(tc.tile_pool(name="p5wb1", bufs=WO_B1))
            p5wb2 = _es35.enter_context(tc.tile_pool(name="p5wb2", bufs=WO_B2))
            with ExitStack() as _es3:
                a_kT = _es3.enter_context(tc.tile_pool(name="a_kT", bufs=3))
                a_q = _es3.enter_context(tc.tile_pool(name="a_q", bufs=1))
                a_v = _es3.enter_context(tc.tile_pool(name="a_v", bufs=2))
                a_p = _es3.enter_context(tc.tile_pool(name="a_p", bufs=2))
                a_misc = _es3.enter_context(tc.tile_pool(name="a_misc", bufs=2))
                a_ps = _es3.enter_context(tc.tile_pool(name="a_ps", bufs=2, space="PSUM"))
                a_vps = _es3.enter_context(tc.tile_pool(name="a_vps", bufs=2, space="PSUM"))
                a_pps = _es3.enter_context(tc.tile_pool(name="a_pps", bufs=2, space="PSUM"))
                a_po = _es3.enter_context(tc.tile_pool(name="a_po", bufs=1, space="PSUM"))

                # A2 wo release: fills the A2A1 window (gated on phase-2 flag)
                wo_gate_release(range(NG0, NG1), p5wa2, flag_p2[:1, :1])

                # fresh-token rows & their transposed columns (once per core)
                kvpe_l = a_misc.tile([BL, CKV], F16, tag="kvpe_l", bufs=1)
                nc.gpsimd.dma_start(kvpe_l[:], a2a1_out[0, :, HL, :])
                warm_burst(kvpe_l[:4, :1])
                kvpeT = a_misc.tile([128, 5 * BL], F16, tag="kvpeT", bufs=1)
                kvps = a_vps.tile([128, 512], F16, tag="vps")
                for ct in range(5):
                    cw = 128 if ct < 4 else DR
                    nc.tensor.transpose(
                        kvps[:cw, ct * BL:(ct + 1) * BL],
                        kvpe_l[:, ct * 128:ct * 128 + cw],
                        identh[:BL, :BL])
                nc.vector.tensor_copy(kvpeT[:, :4 * BL], kvps[:, :4 * BL])
                nc.vector.tensor_copy(kvpeT[:DR, 4 * BL:], kvps[:DR, 4 * BL:5 * BL])

                # first wo release of the attention phase
                wo_gate_release(range(NGC[0], NGC[1]), p5wc, kvpe_l[:1, :1])

                # pre-gather q for ALL batches (keeps the per-batch loop free
                # of gpsimd deps so the mid-loop collective can't stall it)
                qbs, qTs = [], []
                for bl in range(BL):
                    qb = a_q.tile([H, CKV], F16, tag=f"qb{bl}")
                    nc.gpsimd.dma_start(qb[:], a2a1_out[:, bl, :HL, :])
                    qT = a_q.tile([128, 5 * H], F16, tag=f"qT{bl}")
                    qps = a_vps.tile([128, 512], F16, tag="vps")
                    for ct in range(4):
                        nc.tensor.transpose(
                            qps[:, ct * H:(ct + 1) * H],
                            qb[:, ct * 128:(ct + 1) * 128],
                            identh[:H, :H])
                    nc.vector.tensor_copy(qT[:, :4 * H], qps[:])
                    qps2 = a_vps.tile([128, 512], F16, tag="vps")
                    nc.tensor.transpose(
                        qps2[:DR, :H],
                        qb[:, 4 * 128:4 * 128 + DR],
                        identh[:H, :H])
                    nc.vector.tensor_copy(qT[:DR, 4 * H:5 * H], qps2[:DR, :H])
                    qbs.append(qb); qTs.append(qT)

                TW = TB * 128  # 1024 t per block
                for bl in range(BL):
                    qT = qTs[bl]
                    ps_o = a_po.tile([H, KVLR], F32, tag="ps_o")
                    sums = a_misc.tile([H, 2 * NTB], F32, tag="sums")

                    for tb in range(NTB):
                        t0 = tb * TW
                        # K^T tiles straight from HBM
                        kt = [a_kT.tile([128, TW], F16, tag=f"kt{c}", name=f"kt{c}")
                              for c in range(4)]
                        for ct in range(4):
                            nc.sync.dma_start(
                                kt[ct][:], kv_t[bl, ct * 128:(ct + 1) * 128, t0:t0 + TW])
                        ktp = a_kT.tile([DR, TW], F16, tag="ktp")
                        nc.sync.dma_start(ktp[:], pe_t[bl, :, t0:t0 + TW])
                        if tb == NTB - 1:
                            # fresh token at t=4095: overwrite last column
                            for ct in range(4):
                                nc.vector.tensor_copy(
                                    kt[ct][:, TW - 1:TW],
                                    kvpeT[:128, ct * BL + bl:ct * BL + bl + 1])
                            nc.vector.tensor_copy(
                                ktp[:, TW - 1:TW],
                                kvpeT[:DR, 4 * BL + bl:4 * BL + bl + 1])

                        # scores for the two 512-t halves
                        p_sb = a_p.tile([H, TW], F16, tag="p_sb")
                        for half in range(2):
                            hs = slice(half * 512, (half + 1) * 512)
                            ps_s = a_ps.tile([H, 512], F32, tag="ps_s")
                            for ct in range(4):
                                nc.tensor.matmul(
                                    ps_s[:], qT[:, ct * H:(ct + 1) * H],
                                    kt[ct][:, hs], start=(ct == 0), stop=False)
                            nc.tensor.matmul(
                                ps_s[:], qT[:DR, 4 * H:5 * H], ktp[:, hs],
                                start=False, stop=True)
                            nc.scalar.activation(
                                p_sb[:, hs], ps_s[:], AF.Exp, scale=SCALE,
                                accum_out=sums[:, tb * 2 + half:tb * 2 + half + 1])

                        # V tiles via PE transpose of the K^T kv tiles;
                        # P^T via PE transpose of exp'd scores
                        vt_blk = a_v.tile([128, TB * KVLR], F16, tag="vt_blk")
                        pts_blk = a_p.tile([128, TB * H], F16, tag="pts_blk")

                        def v_trans(tt):
                            vps = a_vps.tile([128, 512], F16, tag="vps")
                            for ct in range(4):
                                nc.tensor.transpose(
                                    vps[:, ct * 128:(ct + 1) * 128],
                                    kt[ct][:, tt * 128:(tt + 1) * 128],
                                    identh[:, :])
                            if tt % 2 == 0:
                                nc.vector.tensor_copy(
                                    vt_blk[:, tt * KVLR:(tt + 1) * KVLR], vps[:])
                            else:
                                nc.scalar.copy(
                                    vt_blk[:, tt * KVLR:(tt + 1) * KVLR], vps[:])

                        def p_trans(half):
                            pps = a_pps.tile([128, 512], F16, tag="pps")
                            for tt2 in range(4):
                                nc.tensor.transpose(
                                    pps[:, tt2 * 128:(tt2 + 1) * 128],
                                    p_sb[:, half * 512 + tt2 * 128:
                                         half * 512 + (tt2 + 1) * 128],
                                    identh[:, :])
                            nc.vector.tensor_copy(
                                pts_blk[:, half * 4 * H:(half + 1) * 4 * H], pps[:])

                        for tt in range(4):
                            v_trans(tt)
                        p_trans(0)
                        for tt in range(4, TB):
                            v_trans(tt)
                        for tt in range(4):
                            ti = tb * TB + tt
                            nc.tensor.matmul(ps_o[:], pts_blk[:, tt * H:(tt + 1) * H],
                                             vt_blk[:, tt * KVLR:(tt + 1) * KVLR],
                                             start=(ti == 0), stop=(ti == NT - 1))
                        p_trans(1)
                        for tt in range(4, TB):
                            ti = tb * TB + tt
                            nc.tensor.matmul(ps_o[:], pts_blk[:, tt * H:(tt + 1) * H],
                                             vt_blk[:, tt * KVLR:(tt + 1) * KVLR],
                                             start=(ti == 0), stop=(ti == NT - 1))

                    stot = a_misc.tile([H, 1], F32, tag="stot")
                    nc.vector.tensor_reduce(stot[:], sums[:], mybir.AxisListType.X,
                                            mybir.AluOpType.add)
                    nc.vector.reciprocal(stot[:], stot[:])
                    o_sb = a_misc.tile([H, KVLR], F16, tag="o_sb")
                    nc.scalar.activation(o_sb[:], ps_o[:], AF.Copy, scale=stot[:])
                    if not F_SPLIT2:
                        nc.gpsimd.dma_start(a2a2_in[:, bl, :, :], o_sb[:])
                    elif bl < 2:
                        nc.gpsimd.dma_start(a2a2_in_a[:, bl, :, :], o_sb[:])
                    else:
                        nc.gpsimd.dma_start(a2a2_in_b[:, bl - 2, :, :], o_sb[:])
                    if bl == 0:
                        wo_gate_release(range(NGC[1], NGC[2]), p5wc, o_sb[:1, :1])
                    elif bl == 1:
                        wo_gate_release(range(NGC[2], NGC[3]), p5wc, o_sb[:1, :1])
                        if F_SPLIT2:
                            # first half A2A2 overlaps attention of batches 2-3
                            collective("AllToAll", mybir.AluOpType.bypass, RG,
                                       [a2a2_in_a[:].flatten()], [a2a2_out_a[:].flatten()])
                    elif bl == 2:
                        wo_gate_release(range(NGC[3], NG2), p5wb1, o_sb[:1, :1])
                    elif bl == 3:
                        wo_gate_release(range(NG2, NG3), p5wb2, o_sb[:1, :1])

                if F_SPLIT2:
                    collective("AllToAll", mybir.AluOpType.bypass, RG,
                               [a2a2_in_b[:].flatten()], [a2a2_out_b[:].flatten()])
                else:
                    collective("AllToAll", mybir.AluOpType.bypass, RG,
                               [a2a2_in[:].flatten()], [a2a2_out[:].flatten()])

            # ---------------- Phase 4: uv projection -> o_uv^T tiles ----------------
            with tc.tile_pool(name="p4", bufs=3) as p4, \
                 tc.tile_pool(name="p4ps", bufs=2, space="PSUM") as p4ps, \
                 tc.tile_pool(name="p4psT", bufs=2, space="PSUM") as p4psT:
                NCT = KVLR // 128  # 4
                if F_SPLIT2:
                    # per batch-half: gather [16,512] (contiguous partitions),
                    # transpose, partial uv matmuls, scatter into obT columns
                    # (free-dim strides only -- proven-safe patterns)
                    for half, a2a2_out_h in ((0, a2a2_out_a), (1, a2a2_out_b)):
                        oh_ts = []
                        for h in range(HL):
                            oh_t = p4.tile([16, KVLR], F16, tag=f"oh{half}_{h}",
                                           name=f"oh{half}_{h}", bufs=1)
                            nc.gpsimd.dma_start(oh_t[:], a2a2_out_h[:, :, h, :])
                            oh_ts.append(oh_t)
                        for h in range(HL):
                            oh = oh_ts[h][:]
                            ohps = p4psT.tile([128, NCT * 16], F16, tag="ohps")
                            for ct in range(NCT):
                                nc.tensor.transpose(
                                    ohps[:, ct * 16:(ct + 1) * 16],
                                    oh[:, ct * 128:(ct + 1) * 128],
                                    identh[:16, :16])
                            ohh = p4.tile([128, NCT * 16], F16, tag="ohh")
                            nc.vector.tensor_copy(ohh[:], ohps[:])
                            psuv = p4ps.tile([DV, 16], F32, tag="psuv")
                            for ct in range(NCT):
                                nc.tensor.matmul(
                                    psuv[:], wuv[:, (h * NCT + ct) * DV:(h * NCT + ct + 1) * DV],
                                    ohh[:, ct * 16:(ct + 1) * 16],
                                    start=(ct == 0), stop=(ct == NCT - 1))
                            # scatter into global-batch column order r*4+(2*half+bl)
                            dstv = obT_all[:, h * B:(h + 1) * B].rearrange(
                                "p (r bl) -> p r bl", bl=BL)[:, :, 2 * half:2 * half + 2]
                            srcv = psuv[:].rearrange("p (r bl) -> p r bl", bl=2)
                            if h % 2 == 0:
                                nc.vector.tensor_copy(dstv, srcv)
                            else:
                                nc.scalar.copy(dstv, srcv)
                else:
                    oh_ts = []
                    for h in range(HL):
                        oh_t = p4.tile([B, KVLR], F16, tag=f"oh{h}", name=f"oh{h}", bufs=1)
                        nc.gpsimd.dma_start(oh_t[:], a2a2_out[:, :, h, :])
                        oh_ts.append(oh_t)
                    for h in range(HL):
                        oh = oh_ts[h][:]
                        ohps = p4psT.tile([128, NCT * B], F16, tag="ohps")
                        for ct in range(NCT):
                            nc.tensor.transpose(
                                ohps[:, ct * B:(ct + 1) * B],
                                oh[:, ct * 128:(ct + 1) * 128],
                                identh[:B, :B])
                        ohh = p4.tile([128, NCT * B], F16, tag="ohh")
                        nc.vector.tensor_copy(ohh[:], ohps[:])
                        psuv = p4ps.tile([DV, B], F32, tag="psuv")
                        for ct in range(NCT):
                            nc.tensor.matmul(
                                psuv[:], wuv[:, (h * NCT + ct) * DV:(h * NCT + ct + 1) * DV],
                                ohh[:, ct * B:(ct + 1) * B],
                                start=(ct == 0), stop=(ct == NCT - 1))
                        nc.vector.tensor_copy(obT_all[:, h * B:(h + 1) * B], psuv[:])
            if F_DBG:
                nc.gpsimd.dma_start(dbg_obT, obT_all[:])
            # ---------------- Phase 5: partial out = o_uv @ wo^T (my heads) ----------------
            # col-tiled: each quad q has W5[q]//512 col-groups running
            # concurrently; PSUM tile [128, 512] holds all of them
            with tc.tile_pool(name="p5", bufs=3) as p5, \
                 tc.tile_pool(name="p5cyc", bufs=4) as p5cyc, \
                 tc.tile_pool(name="p5ps", bufs=2, space="PSUM") as p5ps:
                rs_bufs = [(rs_in_a, rs_out_a, 0), (rs_in_b, rs_out_b, 4096),
                           (rs_in_c, rs_out_c, 6144)]
                for q in range(NQUAD):
                    ncg = W5[q] // 512
                    pw = min(ncg, 4) * 32       # psum partition rows used
                    pcw = 512 * ((ncg + 3) // 4)  # psum cols (1 or 2 banks)
                    ps5 = p5ps.tile([128, 1024], F32, tag="ps5")
                    for k in range(NK5):
                        ti = q * NK5 + k
                        if ti in wo_tiles:
                            wt = wo_tiles[ti]
                        else:
                            wt = wo_tile_dma(ti, p5cyc)
                        for n in range(ncg):
                            j, m = n % 4, n // 4
                            nc.tensor.matmul(
                                ps5[32 * j:32 * j + B, m * 512:(m + 1) * 512],
                                obT_all[:, k * B:(k + 1) * B],
                                wt[:, n * 512:(n + 1) * 512],
                                start=(k == 0), stop=(k == NK5 - 1),
                                tile_position=(0, 32 * j))
                    so = p5.tile([128, 1024], F16, tag="so")
                    nc.vector.tensor_copy(so[:pw, :pcw], ps5[:pw, :pcw])
                    dst, rso, off = rs_bufs[q]
                    for n in range(ncg):
                        j, m = n % 4, n // 4
                        nc.gpsimd.dma_start(
                            dst[:, n * 512:(n + 1) * 512],
                            so[32 * j:32 * j + B, m * 512:(m + 1) * 512])
                    collective("ReduceScatter", mybir.AluOpType.add, RG,
                               [dst[:].flatten()], [rso[:].flatten()])
                for q in range(NQUAD):
                    dst, rso, off = rs_bufs[q]
                    nc.gpsimd.dma_start(out_part[:, off:off + W5[q]], rso[:])

    nc.compile()
    return nc


def _get_nc():
    if "nc" not in _CACHE:
        _CACHE["nc"] = _build()
    return _CACHE["nc"]


def _make_in_maps(x, freqs_cos, freqs_sin, kv_cache, pe_cache, wq_a, q_norm_w,
                  wq_b, wkv_a, kv_norm_w, wkv_b, wo):
    f16 = np.float16
    # x^T pre-tiled to the SBUF layout: x_p[p, k*B + b] = x[b, k*128 + p]
    x2 = np.ascontiguousarray(
        np.asarray(x, dtype=np.float32).reshape(B, DIM).T
        .reshape(DIM // 128, 128, B).transpose(1, 0, 2).reshape(128, -1)
        .astype(f16))
    wq_a_t = np.asarray(wq_a, dtype=np.float32).T    # [DIM, QLR] view
    wkv_a_t = np.asarray(wkv_a, dtype=np.float32).T  # [DIM, KVLR+DR] view
    # fold the rmsnorm elementwise weight into wq_b
    wq_b_np = (np.asarray(wq_b, dtype=np.float32)
               * np.asarray(q_norm_w, dtype=np.float32).reshape(1, QLR))
    wkv_b_r = np.asarray(wkv_b, dtype=np.float32).reshape(H, DN + DV, KVLR)
    wo_T = np.asarray(wo, dtype=np.float32).T        # [H*DV, DIM] view
    fc = np.ascontiguousarray(np.broadcast_to(
        np.asarray(freqs_cos, dtype=np.float32).reshape(1, DR // 2), (B, DR // 2)))
    fs = np.ascontiguousarray(np.broadcast_to(
        np.asarray(freqs_sin, dtype=np.float32).reshape(1, DR // 2), (B, DR // 2)))
    knw = np.ascontiguousarray(np.broadcast_to(
        np.asarray(kv_norm_w, dtype=np.float32).reshape(1, KVLR), (B, KVLR)))
    kv_np = np.asarray(kv_cache, dtype=np.float32)
    pe_np = np.asarray(pe_cache, dtype=np.float32)

    in_maps = []
    for r in range(NCORES):
        hs = slice(r * HL, (r + 1) * HL)
        bs = slice(r * BL, (r + 1) * BL)
        # [wq_a^T | wkv_a^T] column slice, pretiled to the SBUF layout:
        # wqkva_p[p, k*PL + e] = wqkva_t[k*128 + p, e]
        wqkva = np.concatenate(
            [wq_a_t[:, r * QL:(r + 1) * QL], wkv_a_t[:, r * KL:(r + 1) * KL]],
            axis=1)   # [DIM, PL]
        wqkva_p = np.ascontiguousarray(
            wqkva.reshape(NKT, 128, PL).transpose(1, 0, 2).reshape(128, -1)
            .astype(f16))
        # wkv_b nope rows in SBUF layout: wn_p[p, h*KVLR + c] = wkv_b[h, p, c]
        wn_pm = np.ascontiguousarray(
            wkv_b_r[hs, :DN, :].transpose(1, 0, 2).reshape(128, -1).astype(f16))
        in_maps.append({
            "x_p": x2,
            "wqkva_p": wqkva_p,
            "wq_b_t": np.ascontiguousarray(
                wq_b_np[r * HL * QKD:(r + 1) * HL * QKD, :].T.astype(f16)),
            "wn_p": wn_pm,
            # uv weights in SBUF layout: [p, (h*4+ct)*DV + dv] = wkv_b[h, DN+dv, ct*128+p]
            "wuv_p": np.ascontiguousarray(
                wkv_b_r[hs, DN:, :].transpose(0, 2, 1)   # [h, c, dv]
                .reshape(HL, KVLR // 128, 128, DV)        # [h, ct, p, dv]
                .transpose(2, 0, 1, 3).reshape(128, -1).astype(f16)),
            "wo_th": np.ascontiguousarray(wo_T[r * KD:(r + 1) * KD, :].astype(f16)),
            "kv_t": np.ascontiguousarray(kv_np[bs].transpose(0, 2, 1).astype(f16)),
            "pe_t": np.ascontiguousarray(pe_np[bs].transpose(0, 2, 1).astype(f16)),
            "kv_norm_w": knw, "fcos": fc, "fsin": fs,
        })
    return in_maps


def _get_runner():
    """Cached jitted SPMD executable (mirrors bass2jax.run_bass_via_pjrt, but
    reuses one jax.jit object so warm calls skip retracing/recompiling)."""
    if "runner" in _CACHE:
        return _CACHE["runner"]
    import jax
    from concourse import bass2jax
    from jax.experimental.shard_map import shard_map
    from jax.sharding import Mesh, PartitionSpec
    import concourse.mybir as mybir_

    nc = _get_nc()
    bass2jax.install_neuronx_cc_hook()
    part_name = nc.partition_id_tensor.name if nc.partition_id_tensor else None
    in_names, out_names, out_avals = [], [], []
    for alloc in nc.m.functions[0].allocations:
        if not isinstance(alloc, mybir_.MemoryLocationSet):
            continue
        name = alloc.memorylocations[0].name
        if alloc.kind == "ExternalInput":
            if name != part_name:
                in_names.append(name)
        elif alloc.kind == "ExternalOutput":
            out_names.append(name)
            out_avals.append(jax.core.ShapedArray(
                tuple(alloc.tensor_shape), mybir_.dt.np(alloc.dtype)))
    n_params = len(in_names)
    all_names = in_names + out_names + ([part_name] if part_name else [])

    def _body(*args):
        operands = list(args)
        if part_name:
            operands.append(bass2jax.partition_id_tensor())
        outs = bass2jax._bass_exec_p.bind(
            *operands, out_avals=tuple(out_avals), in_names=tuple(all_names),
            out_names=tuple(out_names), lowering_input_output_aliases=(),
            sim_require_finite=True, sim_require_nnan=True, nc=nc)
        return tuple(outs)

    devices = jax.devices()[:NCORES]
    mesh = Mesh(np.asarray(devices), ("core",))
    n_outs = len(out_names)
    donate = tuple(range(n_params, n_params + n_outs))
    sharded = jax.jit(
        shard_map(_body, mesh=mesh,
                  in_specs=(PartitionSpec("core"),) * (n_params + n_outs),
                  out_specs=(PartitionSpec("core"),) * n_outs,
                  check_rep=False),
        donate_argnums=donate, keep_unused=True)
    _CACHE["runner"] = (sharded, in_names, out_names, out_avals)
    return _CACHE["runner"]


def _run(in_maps):
    sharded, in_names, out_names, out_avals = _get_runner()
    concat_in = [np.concatenate([in_maps[c][n] for c in range(NCORES)], axis=0)
                 for n in in_names]
    concat_zeros = [np.zeros((NCORES * a.shape[0], *a.shape[1:]), a.dtype)
                    for a in out_avals]
    out_arrs = sharded(*concat_in, *concat_zeros)
    return {n: np.asarray(out_arrs[i]) for i, n in enumerate(out_names)}


def _assemble(parts):
    # parts[r] is [BL, DIM] (fp16) for batches r*BL .. (r+1)*BL-1
    out = np.concatenate(
        [np.asarray(p).astype(np.float32).reshape(BL, DIM) for p in parts], axis=0)
    return np.ascontiguousarray(out.reshape(B, 1, DIM))


def kernel(x, freqs_cos, freqs_sin, kv_cache, pe_cache, wq_a, q_norm_w,
           wq_b, wkv_a, kv_norm_w, wkv_b, wo, start_pos, _trace=False):
    assert int(start_pos) == SPOS, f"kernel compiled for start_pos={SPOS}"
    in_maps = _make_in_maps(x, freqs_cos, freqs_sin, kv_cache, pe_cache, wq_a,
                            q_norm_w, wq_b, wkv_a, kv_norm_w, wkv_b, wo)
    if _trace:
        import tempfile
        tmpdir = tempfile.mkdtemp(prefix="mla_trace_")
        res = run_bass_kernel_spmd(_get_nc(), in_maps,
                                   core_ids=list(range(NCORES)),
                                   trace=True, tmpdir=tmpdir)
        out = _assemble([res.results[r]["out_part"] for r in range(NCORES)])
        print(f"trace tmpdir: {tmpdir}")
        if res.instructions_and_trace is not None:
            print(f"trace path: {res.instructions_and_trace[1]}")
        return out, res
    outs = _run(in_maps)
    return _assemble(outs["out_part"].reshape(NCORES, BL, DIM))


# revision 25
# speedup vs baseline: 1.0394x; 1.0394x over previous
"""MLA decode (DeepSeek-V3 dims, absorbed attention) on 8 Trainium2 NeuronCores.

Sharding (v6):
  - wq_a/wkv_a output-sharded (192/72 dims per core) -> AllGather [32,264]
  - wq_b / wkv_b head-sharded (16 heads per core); q_abs absorbed on producer
  - AllToAll #1 redistributes q (+ fresh kv/pe token) to batch-sharded layout
  - attention batch-sharded (4 batches per core); caches host-pretransposed to
    [c, t]; V tiles rebuilt via PE transposes; softmax without max-subtraction
  - AllToAll #2 (split in two batch-halves for overlap) back to head-sharded
    for the uv projection -> o_uv^T tiles
  - wo column-sharded by local heads -> partial [32, 7168] finished with a
    two-part ReduceScatter (split by output columns for overlap)
v11 changes vs v5 (baseline 479.6us -> ~405-425us):
  - wqkva and wkv_b_nope pretiled on host -> contiguous line-rate DMAs
  - wq_b fully prefetched during phase 1 / AllGather window
  - dummy warm-up collective at t=0 absorbs the ~40-50us cc-stream init
    barrier concurrently with phase 1 (the real AllGather then runs in ~5us)
  - PE warm-up bursts gated on collective outputs: force the HAM clock gate
    back to full rate before each compute phase (PE transposes don't count
    as activity, so phases after a stall otherwise run at 1.2 GHz)
  - col-tiled (tile_position) matmuls for the M=32 GEMMs (phase 2 and the
    wo projection): 3-4 subtiles run concurrently on the PE array
  - wo tiles grown to [128, 4096/2048/1024] (up to 1 MB DMAs) on the Scalar
    HWDGE queue so they never block the Sync queue that feeds attention
  - AllToAll #2 split in two batch-halves (first overlaps attention of
    batches 2-3; phase 4 is split to match), ReduceScatter split in three
    column pieces so only the last ~64 KB piece is exposed at the tail
  - phase 2 restructured: both head-groups' matmuls run before rope/absorb
  - NOTE: gpsimd DMAs with partition-dim splits or strided partition
    subsets on the SBUF side silently corrupt data (two debugging rounds);
    only full-range partition APs + free-dim strides are used
All heavy streams are fp16; accumulations, norms and softmax stats are fp32.
"""
from contextlib import ExitStack

import numpy as np

import concourse.bacc as bacc
import concourse.tile as tile
import concourse.mybir as mybir
from concourse.bass_utils import run_bass_kernel_spmd
from concourse import masks

F32 = mybir.dt.float32
F16 = mybir.dt.float16
AF = mybir.ActivationFunctionType

DIM = 7168; H = 128; QLR = 1536; KVLR = 512
DN = 128; DR = 64; DV = 128; QKD = DN + DR
B = 32; MAXS = 4096; SPOS = 4095
SCALE = QKD ** -0.5
EPS = 1e-6

NCORES = 8
HL = H // NCORES          # 16 local heads
BL = B // NCORES          # 4 local batches
QL = QLR // NCORES        # 192 wq_a out dims per core
KL = (KVLR + DR) // NCORES  # 72 wkv_a out dims per core
PL = QL + KL              # 264 phase-1 out dims per core
CKV = KVLR + DR           # 576
NKT = DIM // 128          # 56 k-tiles of phase 1
NT = MAXS // 128          # 32 t-tiles
TB = 8                    # t-tiles per block
NTB = NT // TB            # 4 blocks per batch
KD = HL * DV              # 2048 contraction dims of wo per core
NK5 = KD // 128           # 16 wo k-tiles
# phase-5 wo tiling: quad widths (sum = DIM); tiles [128, W5[q]]
W5 = [4096, 2048, 1024]
W5OFF = [0, 4096, 6144]
NQUAD = len(W5)
# prefetched wo tiles (consumption order ti = q*NK5 + k):
WO_A, WO_A2, WO_C, WO_B1, WO_B2 = 4, 2, 4, 2, 2

_CACHE = {}

# feature flags (kept for debugging; production values below)
F_POKE = 1    # PE warm-up chain + HAM warm bursts after collectives
F_COLT2 = 1   # col-tiled (tile_position) phase-2 matmuls
F_COLT5 = 1   # col-tiled wo projection
F_SPLIT = 1   # (legacy) split tail collectives
F_DBG = int(__import__('os').environ.get('F_DBG', '0'))
F_A2A2V5 = 1  # single-buffer A2A2 layout in the non-split path
F_SPLIT2 = 1  # A2A2 split in two batch-halves, phase 4 split to match


def _build(spmd=True):
    nc = bacc.Bacc("TRN2", target_bir_lowering=False, debug=False,
                   enable_asserts=False, num_devices=NCORES if spmd else 1)

    def collective(kind, op, replica_groups, ins, outs):
        if spmd:
            nc.gpsimd.collective_compute(kind, op, replica_groups=replica_groups,
                                         ins=ins, outs=outs)
            return
        i_ap, o_ap = ins[0], outs[0]
        ni = i_ap.shape[0]
        if kind == "AllGather":
            for r in range(o_ap.shape[0] // ni):
                nc.sync.dma_start(o_ap[r * ni:(r + 1) * ni], i_ap)
        elif kind == "ReduceScatter":
            nc.sync.dma_start(o_ap, i_ap[:o_ap.shape[0]])
        else:
            nc.sync.dma_start(o_ap, i_ap)
    din = {}
    def inp(name, shape, dt=F16):
        din[name] = nc.dram_tensor(name, list(shape), dt, kind="ExternalInput").ap()
        return din[name]

    x_p = inp("x_p", [128, NKT * B])                 # x^T pre-tiled to SBUF layout
    wqkva_p = inp("wqkva_p", [128, NKT * PL])        # [wq_a^T|wkv_a^T] SBUF layout
    wq_b_t = inp("wq_b_t", [QLR, HL * QKD])          # wq_b^T col slice (my heads)
    wn_p = inp("wn_p", [128, HL * KVLR])             # wkv_b nope rows, SBUF layout
    wuv_p = inp("wuv_p", [128, HL * (KVLR // 128) * DV])  # uv weights, SBUF layout
    wo_th = inp("wo_th", [KD, DIM])                  # wo^T rows for my heads
    kv_t = inp("kv_t", [BL, KVLR, MAXS])             # latent cache^T, my batches
    pe_t = inp("pe_t", [BL, DR, MAXS])               # pe cache^T, my batches
    kv_norm_w = inp("kv_norm_w", [B, KVLR], F32)
    fcos = inp("fcos", [B, DR // 2], F32)
    fsin = inp("fsin", [B, DR // 2], F32)

    out_part = nc.dram_tensor("out_part", [BL, DIM], F16, kind="ExternalOutput").ap()
    if F_DBG:
        dbg_qlr = nc.dram_tensor("dbg_qlr", [B, QLR], F32, kind="ExternalOutput").ap()
        dbg_qsb = nc.dram_tensor("dbg_qsb", [B, HL * QKD], F32, kind="ExternalOutput").ap()
        dbg_obT = nc.dram_tensor("dbg_obT", [128, HL * B], F16, kind="ExternalOutput").ap()

    RG = [list(range(NCORES))]

    with tile.TileContext(nc) as tc:
        with ExitStack() as _es:
            cpool = _es.enter_context(tc.tile_pool(name="const", bufs=1))
            dram = _es.enter_context(tc.tile_pool(name="dram", bufs=1, space="DRAM"))
            p5wa = _es.enter_context(tc.tile_pool(name="p5wa", bufs=WO_A))
            p4w = _es.enter_context(tc.tile_pool(name="p4w", bufs=1))
            p5keep = _es.enter_context(tc.tile_pool(name="p5keep", bufs=1))
            flags = _es.enter_context(tc.tile_pool(name="flags", bufs=2))
            warmps = _es.enter_context(tc.tile_pool(name="warmps", bufs=1, space="PSUM"))
            ident = cpool.tile([128, 128], F32)
            masks.make_identity(nc, ident[:])
            identh_t = cpool.tile([128, 128], F16)
            nc.vector.tensor_copy(identh_t[:], ident[:])
            identh = identh_t[:]

            # PE warm-up chain: back-to-back matmuls on the identity while the
            # first DMAs stream, so HAM un-throttles before phase-1 math
            wps = warmps.tile([128, 512], F32)
            if F_POKE:
                for wi in range(50):
                    nc.tensor.matmul(wps[:, :128], identh[:, :], identh[:, :],
                                     start=True, stop=True)

            def poke(src_ap):
                if F_POKE:
                    nc.tensor.matmul(wps[:1, :1], src_ap, src_ap, start=True, stop=True)

            def warm_burst(dep_ap, n=36):
                # ~3.4us of back-to-back matmuls gated on dep_ap's producer:
                # forces the HAM activity monitor back to full clock before a
                # compute phase starts (PE transposes alone don't count)
                if F_POKE:
                    nc.tensor.matmul(wps[:1, :1], dep_ap, dep_ap, start=True, stop=True)
                    for _ in range(n):
                        nc.tensor.matmul(wps[:, :128], identh[:, :], identh[:, :],
                                         start=True, stop=True)

            # persistent collective DRAM buffers
            agw_in = dram.tile([1, 8], F32)
            agw_out = dram.tile([NCORES, 8], F32, addr_space="Shared" if spmd else "Local")
            ag1_in = dram.tile([B, PL], F32)
            ag1_out = dram.tile([NCORES, B, PL], F32, addr_space="Shared" if spmd else "Local")
            a2a1_in = dram.tile([NCORES, BL, HL + 1, CKV], F16)
            a2a1_out = dram.tile([NCORES, BL, HL + 1, CKV], F16)
            if F_SPLIT2:
                a2a2_in_a = dram.tile([NCORES, 2, HL, KVLR], F16)
                a2a2_out_a = dram.tile([NCORES, 2, HL, KVLR], F16)
                a2a2_in_b = dram.tile([NCORES, 2, HL, KVLR], F16)
                a2a2_out_b = dram.tile([NCORES, 2, HL, KVLR], F16)
            else:
                a2a2_in = dram.tile([NCORES, BL, HL, KVLR], F16)
                a2a2_out = dram.tile([NCORES, BL, HL, KVLR], F16)
            rs_in_a = dram.tile([B, 4096], F16)
            rs_out_a = dram.tile([BL, 4096], F16)
            rs_in_b = dram.tile([B, 2048], F16)
            rs_out_b = dram.tile([BL, 2048], F16)
            rs_in_c = dram.tile([B, 1024], F16)
            rs_out_c = dram.tile([BL, 1024], F16)

            # o_uv^T tiles, produced by phase 4, consumed by phase 5
            obT_all = p5keep.tile([128, HL * B], F16)

            # ---- wo prefetch tiles; all on the Scalar HWDGE queue so they
            # never head-of-line-block the Sync queue (kv/attention stream).
            # Consumption order ti = q*NK5 + k (quad-major).
            wo_tiles = {}

            # warm-up collective: absorbs the cc-stream init barrier (~40-50us)
            # concurrently with phase 1 instead of serializing before the
            # first real AllGather
            warm_sb = cpool.tile([1, 8], F32)
            nc.gpsimd.memset(warm_sb[:], 0.0)
            nc.gpsimd.dma_start(agw_in[:], warm_sb[:])
            if spmd:
                collective("AllGather", mybir.AluOpType.bypass, RG,
                           [agw_in[:].flatten()], [agw_out[:].flatten()])

            def wo_tile_dma(ti, pool):
                q, k = divmod(ti, NK5)
                wt = pool.tile([128, W5[q]], F16, tag=f"p5w{W5[q]}")
                wo_tiles[ti] = wt
                nc.scalar.dma_start(
                    wt[:], wo_th[k * 128:(k + 1) * 128,
                                 W5OFF[q]:W5OFF[q] + W5[q]])
                return wt

            def wo_gate_release(tis, pool, src_ap, do_poke=False):
                # gpsimd writes one element of each tile (WAW) right after
                # src_ap's producer, then the DMA may start
                for ti in tis:
                    q, k = divmod(ti, NK5)
                    wt = pool.tile([128, W5[q]], F16, tag=f"p5w{W5[q]}")
                    wo_tiles[ti] = wt
                    nc.gpsimd.tensor_copy(wt[:1, :1], src_ap)
                    nc.scalar.dma_start(
                        wt[:], wo_th[k * 128:(k + 1) * 128,
                                     W5OFF[q]:W5OFF[q] + W5[q]])
                    if do_poke:
                        poke(wt[:32, :1])

            NG0 = WO_A
            NG1 = NG0 + WO_A2
            NGC = [NG1, NG1 + 2, NG1 + 3, NG1 + WO_C]  # C released in 3 steps
            NG2 = NG1 + WO_C + WO_B1
            NG3 = NG2 + WO_B2

            # ================= Phases 1-2 scope =================
            with ExitStack() as _es12:
                p2wq = _es12.enter_context(tc.tile_pool(name="p2wq", bufs=12))
                pb = _es12.enter_context(tc.tile_pool(name="p1b", bufs=1))
                p2wn = _es12.enter_context(tc.tile_pool(name="p2wn", bufs=1))

                # ---------------- Phase 1: q_lr & kv_full partials ----------------
                KCH = 14                       # k-tiles per wqkva chunk
                with tc.tile_pool(name="p1sb", bufs=1) as p1sb, \
                     tc.tile_pool(name="p1w", bufs=4) as p1w, \
                     tc.tile_pool(name="p1ps", bufs=1, space="PSUM") as p1ps:
                    xT = p1sb.tile([128, NKT * B], F16)
                    nc.sync.dma_start(xT[:], x_p[:])
                    wch = []
                    for c4 in range(NKT // KCH):
                        wt = p1w.tile([128, KCH * PL], F16, tag="p1w")
                        nc.sync.dma_start(
                            wt[:], wqkva_p[:, c4 * KCH * PL:(c4 + 1) * KCH * PL])
                        wch.append(wt)
                    # wq_b prefetch: stream all 12 row-blocks during phase1+AG
                    wqb_tiles = []
                    for k in range(QLR // 128):
                        wt = p2wq.tile([128, HL * QKD], F16, tag="p2wq")
                        nc.sync.dma_start(wt[:], wq_b_t[k * 128:(k + 1) * 128, :])
                        wqb_tiles.append(wt)
                    ps1 = p1ps.tile([B, PL], F32)
                    for c4 in range(NKT // KCH):
                        for j in range(KCH):
                            k = c4 * KCH + j
                            nc.tensor.matmul(ps1[:], xT[:, k * B:(k + 1) * B],
                                             wch[c4][:, j * PL:(j + 1) * PL],
                                             start=(k == 0), stop=(k == NKT - 1))
                    st1 = p1sb.tile([B, PL], F32)
                    nc.vector.tensor_copy(st1[:], ps1[:])
                    nc.gpsimd.dma_start(ag1_in[:], st1[:])
                    collective("AllGather", mybir.AluOpType.bypass, RG,
                               [ag1_in[:].flatten()], [ag1_out[:].flatten()])


                # wn / wuv / wo-A gated on the last wq_b tile so they never
                # compete with it for HBM bandwidth
                wn_all = p2wn.tile([128, HL * KVLR], F16, tag="wn")
                nc.gpsimd.tensor_copy(wn_all[:1, :1], wqb_tiles[-1][:1, :1])
                nc.scalar.dma_start(wn_all[:], wn_p[:])
                wuv = p4w.tile([128, HL * (KVLR // 128) * DV], F16, tag="wuv")
                nc.gpsimd.tensor_copy(wuv[:1, :1], wqb_tiles[-1][:1, :1])
                nc.scalar.dma_start(wuv[:], wuv_p[:])
                wo_gate_release(range(NG0), p5wa, wqb_tiles[-1][:1, :1])

                # ---------------- Phase 1b: rmsnorm + rope (all batches) ----------------
                q_lr = pb.tile([B, QLR], F32)
                # gather [32, 8, 192] -> [32, 1536]
                nc.gpsimd.dma_start(
                    q_lr[:].rearrange("b (r q) -> b r q", r=NCORES),
                    ag1_out[:, :, :QL].rearrange("r b q -> b r q"))
                kvf = pb.tile([B, CKV], F32)
                nc.gpsimd.dma_start(
                    kvf[:].rearrange("b (r q) -> b r q", r=NCORES),
                    ag1_out[:, :, QL:].rearrange("r b q -> b r q"))
                warm_burst(q_lr[:32, :1])

                # rmsnorm(q_lr): elementwise weight folded into wq_b on host;
                # the 1/rms scale is applied to phase-2 matmul outputs.
                eps_t = pb.tile([B, 1], F32)
                nc.gpsimd.memset(eps_t[:], EPS)
                qs = pb.tile([B, 1], F32)
                sq_tmp = pb.tile([B, QLR], F32)
                nc.scalar.activation(sq_tmp[:], q_lr[:], AF.Square, accum_out=qs[:])
                nc.scalar.activation(qs[:], qs[:], AF.Sqrt, scale=1.0 / QLR, bias=eps_t[:])
                nc.vector.reciprocal(qs[:], qs[:])
                if F_DBG:
                    nc.gpsimd.dma_start(dbg_qlr, q_lr[:])

                # q_lr^T tiles for phase 2
                qlrT = pb.tile([128, (QLR // 128) * B], F16)
                with tc.tile_pool(name="p1bps", bufs=2, space="PSUM") as pbps:
                    for k in range(QLR // 128):
                        pT = pbps.tile([128, B], F32, tag="p1bT")
                        nc.tensor.transpose(pT[:], q_lr[:, k * 128:(k + 1) * 128], ident[:B, :B])
                        nc.vector.tensor_copy(qlrT[:, k * B:(k + 1) * B], pT[:])

                # kvpe_new = [rmsnorm(kv) | rope(k_pe)]
                kvpe_new = pb.tile([B, CKV], F32)
                ks = pb.tile([B, 1], F32)
                kv_tmp = pb.tile([B, KVLR], F32)
                nc.scalar.activation(kv_tmp[:], kvf[:, :KVLR], AF.Square, accum_out=ks[:])
                nc.scalar.activation(ks[:], ks[:], AF.Sqrt, scale=1.0 / KVLR, bias=eps_t[:])
                nc.vector.reciprocal(ks[:], ks[:])
                nc.vector.tensor_copy(kvpe_new[:, :KVLR], kvf[:, :KVLR])
                nc.vector.tensor_scalar_mul(kvpe_new[:, :KVLR], kvpe_new[:, :KVLR], ks[:])
                knw = pb.tile([B, KVLR], F32)
                nc.sync.dma_start(knw[:], kv_norm_w[:])
                nc.vector.tensor_tensor(kvpe_new[:, :KVLR], kvpe_new[:, :KVLR],
                                        knw[:], mybir.AluOpType.mult)

                cosb = pb.tile([B, DR // 2], F32)
                sinb = pb.tile([B, DR // 2], F32)
                nc.sync.dma_start(cosb[:], fcos[:])
                nc.sync.dma_start(sinb[:], fsin[:])

                pe_src = kvf[:, KVLR:].rearrange("b (i two) -> b i two", two=2)
                pe_dst = kvpe_new[:, KVLR:].rearrange("b (i two) -> b i two", two=2)
                t1r = pb.tile([B, DR // 2], F32)
                t2r = pb.tile([B, DR // 2], F32)
                x1, x2 = pe_src[:, :, 0], pe_src[:, :, 1]
                nc.vector.tensor_tensor(t1r[:], x1, cosb[:], mybir.AluOpType.mult)
                nc.vector.tensor_tensor(t2r[:], x2, sinb[:], mybir.AluOpType.mult)
                nc.vector.tensor_tensor(pe_dst[:, :, 0], t1r[:], t2r[:], mybir.AluOpType.subtract)
                nc.vector.tensor_tensor(t1r[:], x1, sinb[:], mybir.AluOpType.mult)
                nc.vector.tensor_tensor(t2r[:], x2, cosb[:], mybir.AluOpType.mult)
                nc.vector.tensor_tensor(pe_dst[:, :, 1], t1r[:], t2r[:], mybir.AluOpType.add)

                # send fresh kv/pe rows through A2A slot h==HL (fp16 payload)
                kvpe_h = pb.tile([B, CKV], F16)
                nc.vector.tensor_copy(kvpe_h[:], kvpe_new[:])
                nc.gpsimd.dma_start(a2a1_in[:, :, HL, :], kvpe_h[:])

                # ---------------- Phase 2: q = q_lr_n @ wq_b^T, rope, absorb ----------------
                # col-tiled: 3 n-blocks of 512 run concurrently per k-tile
                with tc.tile_pool(name="p2", bufs=1) as p2, \
                     tc.tile_pool(name="p2st", bufs=3) as p2st:
                    NQ = HL * QKD  # 3072
                    HG = HL // 2   # heads per group
                    NQG = HG * QKD  # 1536 columns per group
                    q_sb = p2.tile([B, NQ], F32)
                    rope_q = p2.tile([B, HL, DR], F32)
                    rope_q2 = rope_q[:].rearrange("b h (i two) -> b h i two", two=2)
                    cb = cosb[:].rearrange("b (h i) -> b h i", h=1).to_broadcast((B, HG, DR // 2))
                    sb_ = sinb[:].rearrange("b (h i) -> b h i", h=1).to_broadcast((B, HG, DR // 2))
                    t1 = p2.tile([B, HL * DR // 2], F32)
                    t2 = p2.tile([B, HL * DR // 2], F32)

                    with tc.tile_pool(name="p2ps", bufs=2, space="PSUM") as p2ps, \
                         tc.tile_pool(name="p2ps2", bufs=2, space="PSUM") as p2ps2:
                      flag_p2 = flags.tile([1, 1], F16, tag="flag_p2")
                      # 1) both groups' q matmuls first (keeps TensorE dense)
                      psqs = []
                      for g in range(2):
                        gc0 = g * NQG
                        psq = p2ps.tile([128, 512], F32, tag="psq", name=f"psq{g}")
                        psqs.append(psq)
                        for k in range(QLR // 128):
                            for n in range(NQG // 512):
                                nc.tensor.matmul(
                                    psq[32 * n:32 * n + B, :],
                                    qlrT[:, k * B:(k + 1) * B],
                                    wqb_tiles[k][:, gc0 + n * 512:gc0 + (n + 1) * 512],
                                    start=(k == 0), stop=(k == QLR // 128 - 1),
                                    tile_position=(0, 32 * n))
                        for n in range(NQG // 512):
                            nc.vector.tensor_scalar_mul(
                                q_sb[:, gc0 + n * 512:gc0 + (n + 1) * 512],
                                psq[32 * n:32 * n + B, :], qs[:])

                      # 2) rope all heads in one pass
                      cba = cosb[:].rearrange("b (h i) -> b h i", h=1).to_broadcast((B, HL, DR // 2))
                      sba = sinb[:].rearrange("b (h i) -> b h i", h=1).to_broadcast((B, HL, DR // 2))
                      qpe2a = q_sb[:].rearrange("b (h d) -> b h d", h=HL)[:, :, DN:] \
                          .rearrange("b h (i two) -> b h i two", two=2)
                      t1v = t1[:].rearrange("b (h i) -> b h i", h=HL)
                      t2v = t2[:].rearrange("b (h i) -> b h i", h=HL)
                      x1, x2 = qpe2a[:, :, :, 0], qpe2a[:, :, :, 1]
                      nc.vector.tensor_tensor(t1v, x1, cba, mybir.AluOpType.mult)
                      nc.vector.tensor_tensor(t2v, x2, sba, mybir.AluOpType.mult)
                      nc.vector.tensor_tensor(rope_q2[:, :, :, :, 0] if False else rope_q2[:, :, :, 0], t1v, t2v, mybir.AluOpType.subtract)
                      nc.vector.tensor_tensor(t1v, x1, sba, mybir.AluOpType.mult)
                      nc.vector.tensor_tensor(t2v, x2, cba, mybir.AluOpType.mult)
                      nc.vector.tensor_tensor(rope_q2[:, :, :, 1], t1v, t2v, mybir.AluOpType.add)

                      # 3) absorb quads of heads; alternate copy engines
                      for hp_g in range(HL // 4):
                            stage = p2st.tile([B, 4 * CKV], F16, tag="stage")
                            for j in range(4):
                                h = 4 * hp_g + j
                                qnT = p2ps2.tile([DN, B], F32, tag="qnT")
                                nc.tensor.transpose(
                                    qnT[:], q_sb[:, h * QKD:h * QKD + DN],
                                    ident[:B, :B])
                                qnTs = p2st.tile([DN, B], F16, tag="qnTs")
                                pabs = p2ps2.tile([B, KVLR], F32, tag="pabs")
                                if j % 2 == 0:
                                    nc.vector.tensor_copy(qnTs[:], qnT[:])
                                else:
                                    nc.scalar.copy(qnTs[:], qnT[:])
                                nc.tensor.matmul(pabs[:], qnTs[:],
                                                 wn_all[:, h * KVLR:(h + 1) * KVLR],
                                                 start=True, stop=True)
                                if j % 2 == 0:
                                    nc.vector.tensor_copy(
                                        stage[:, j * CKV:j * CKV + KVLR], pabs[:])
                                    nc.vector.tensor_copy(
                                        stage[:, j * CKV + KVLR:(j + 1) * CKV],
                                        rope_q[:, h, :])
                                else:
                                    nc.scalar.copy(
                                        stage[:, j * CKV:j * CKV + KVLR], pabs[:])
                                    nc.scalar.copy(
                                        stage[:, j * CKV + KVLR:(j + 1) * CKV],
                                        rope_q[:, h, :])
                            nc.gpsimd.dma_start(
                                a2a1_in[:, :, 4 * hp_g:4 * hp_g + 4, :],
                                stage[:].rearrange("b (j c) -> b j c", j=4))
                            if hp_g == HL // 4 - 1:
                                # flag: A2 wo prefetch releases off this point
                                nc.vector.tensor_copy(flag_p2[:], stage[:1, :1])

                    if F_DBG:
                        nc.gpsimd.dma_start(dbg_qsb, q_sb[:])
                    collective("AllToAll", mybir.AluOpType.bypass, RG,
                               [a2a1_in[:].flatten()], [a2a1_out[:].flatten()])

            # ---------------- Phase 3: attention, batch-sharded ----------------
            _es35 = _es.enter_context(ExitStack())
            p5wa2 = _es35.enter_context(tc.tile_pool(name="p5wa2", bufs=WO_A2))
            p5wc = _es35.enter_context(tc.tile_pool(name="p5wc", bufs=WO_C))
            p5wb1 = _es35.enter_context(tc.tile_pool(name="p5wb1", bufs=WO_B1))
            p5wb2 = _es35.enter_context(tc.tile_pool(name="p5wb2", bufs=WO_B2))
            with ExitStack() as _es3:
                a_kT = _es3.enter_context(tc.tile_pool(name="a_kT", bufs=3))
                a_q = _es3.enter_context(tc.tile_pool(name="a_q", bufs=1))
                a_v = _es3.enter_context(tc.tile_pool(name="a_v", bufs=2))
                a_p = _es3.enter_context(tc.tile_pool(name="a_p", bufs=2))
                a_misc = _es3.enter_context(tc.tile_pool(name="a_misc", bufs=2))
                a_ps = _es3.enter_context(tc.tile_pool(name="a_ps", bufs=2, space="PSUM"))
                a_vps = _es3.enter_context(tc.tile_pool(name="a_vps", bufs=2, space="PSUM"))
                a_pps = _es3.enter_context(tc.tile_pool(name="a_pps", bufs=2, space="PSUM"))
                a_po = _es3.enter_context(tc.tile_pool(name="a_po", bufs=1, space="PSUM"))

                # A2 wo release: fills the A2A1 window (gated on phase-2 flag)
                wo_gate_release(range(NG0, NG1), p5wa2, flag_p2[:1, :1])

                # fresh-token rows & their transposed columns (once per core)
                kvpe_l = a_misc.tile([BL, CKV], F16, tag="kvpe_l", bufs=1)
                nc.gpsimd.dma_start(kvpe_l[:], a2a1_out[0, :, HL, :])
                warm_burst(kvpe_l[:4, :1])
                kvpeT = a_misc.tile([128, 5 * BL], F16, tag="kvpeT", bufs=1)
                kvps = a_vps.tile([128, 512], F16, tag="vps")
                for ct in range(5):
                    cw = 128 if ct < 4 else DR
                    nc.tensor.transpose(
                        kvps[:cw, ct * BL:(ct + 1) * BL],
                        kvpe_l[:, ct * 128:ct * 128 + cw],
                        identh[:BL, :BL])
                nc.vector.tensor_copy(kvpeT[:, :4 * BL], kvps[:, :4 * BL])
                nc.vector.tensor_copy(kvpeT[:DR, 4 * BL:], kvps[:DR, 4 * BL:5 * BL])

                # first wo release of the attention phase
                wo_gate_release(range(NGC[0], NGC[1]), p5wc, kvpe_l[:1, :1])

                # pre-gather q for ALL batches (keeps the per-batch loop free
                # of gpsimd deps so the mid-loop collective can't stall it)
                qbs, qTs = [], []
                for bl in range(BL):
                    qb = a_q.tile([H, CKV], F16, tag=f"qb{bl}")
                    nc.gpsimd.dma_start(qb[:], a2a1_out[:, bl, :HL, :])
                    qT = a_q.tile([128, 5 * H], F16, tag=f"qT{bl}")
                    qps = a_vps.tile([128, 512], F16, tag="vps")
                    for ct in range(4):
                        nc.tensor.transpose(
                            qps[:, ct * H:(ct + 1) * H],
                            qb[:, ct * 128:(ct + 1) * 128],
                            identh[:H, :H])
                    nc.vector.tensor_copy(qT[:, :4 * H], qps[:])
                    qps2 = a_vps.tile([128, 512], F16, tag="vps")
                    nc.tensor.transpose(
                        qps2[:DR, :H],
                        qb[:, 4 * 128:4 * 128 + DR],
                        identh[:H, :H])
                    nc.vector.tensor_copy(qT[:DR, 4 * H:5 * H], qps2[:DR, :H])
                    qbs.append(qb); qTs.append(qT)

                TW = TB * 128  # 1024 t per block
                for bl in range(BL):
                    qT = qTs[bl]
                    ps_o = a_po.tile([H, KVLR], F32, tag="ps_o")
                    sums = a_misc.tile([H, 2 * NTB], F32, tag="sums")

                    for tb in range(NTB):
                        t0 = tb * TW
                        # K^T tiles straight from HBM
                        kt = [a_kT.tile([128, TW], F16, tag=f"kt{c}", name=f"kt{c}")
                              for c in range(4)]
                        for ct in range(4):
                            nc.sync.dma_start(
                                kt[ct][:], kv_t[bl, ct * 128:(ct + 1) * 128, t0:t0 + TW])
                        ktp = a_kT.tile([DR, TW], F16, tag="ktp")
                        nc.sync.dma_start(ktp[:], pe_t[bl, :, t0:t0 + TW])
                        if tb == NTB - 1:
                            # fresh token at t=4095: overwrite last column
                            for ct in range(4):
                                nc.vector.tensor_copy(
                                    kt[ct][:, TW - 1:TW],
                                    kvpeT[:128, ct * BL + bl:ct * BL + bl + 1])
                            nc.vector.tensor_copy(
                                ktp[:, TW - 1:TW],
                                kvpeT[:DR, 4 * BL + bl:4 * BL + bl + 1])

                        # scores for the two 512-t halves
                        p_sb = a_p.tile([H, TW], F16, tag="p_sb")
                        for half in range(2):
                            hs = slice(half * 512, (half + 1) * 512)
                            ps_s = a_ps.tile([H, 512], F32, tag="ps_s")
                            for ct in range(4):
                                nc.tensor.matmul(
                                    ps_s[:], qT[:, ct * H:(ct + 1) * H],
                                    kt[ct][:, hs], start=(ct == 0), stop=False)
                            nc.tensor.matmul(
                                ps_s[:], qT[:DR, 4 * H:5 * H], ktp[:, hs],
                                start=False, stop=True)
                            nc.scalar.activation(
                                p_sb[:, hs], ps_s[:], AF.Exp, scale=SCALE,
                                accum_out=sums[:, tb * 2 + half:tb * 2 + half + 1])

                        # V tiles via PE transpose of the K^T kv tiles;
                        # P^T via PE transpose of exp'd scores
                        vt_blk = a_v.tile([128, TB * KVLR], F16, tag="vt_blk")
                        pts_blk = a_p.tile([128, TB * H], F16, tag="pts_blk")

                        def v_trans(tt):
                            vps = a_vps.tile([128, 512], F16, tag="vps")
                            for ct in range(4):
                                nc.tensor.transpose(
                                    vps[:, ct * 128:(ct + 1) * 128],
                                    kt[ct][:, tt * 128:(tt + 1) * 128],
                                    identh[:, :])
                            if tt % 2 == 0:
                                nc.vector.tensor_copy(
                                    vt_blk[:, tt * KVLR:(tt + 1) * KVLR], vps[:])
                            else:
                                nc.scalar.copy(
                                    vt_blk[:, tt * KVLR:(tt + 1) * KVLR], vps[:])

                        def p_trans(half):
                            pps = a_pps.tile([128, 512], F16, tag="pps")
                            for tt2 in range(4):
                                nc.tensor.transpose(
                                    pps[:, tt2 * 128:(tt2 + 1) * 128],
                                    p_sb[:, half * 512 + tt2 * 128:
                                         half * 512 + (tt2 + 1) * 128],
                                    identh[:, :])
                            nc.vector.tensor_copy(
                                pts_blk[:, half * 4 * H:(half + 1) * 4 * H], pps[:])

                        for tt in range(4):
                            v_trans(tt)
                        p_trans(0)
                        for tt in range(4, TB):
                            v_trans(tt)
                        for tt in range(4):
                            ti = tb * TB + tt
                            nc.tensor.matmul(ps_o[:], pts_blk[:, tt * H:(tt + 1) * H],
                                             vt_blk[:, tt * KVLR:(tt + 1) * KVLR],
                                             start=(ti == 0), stop=(ti == NT - 1))
                        p_trans(1)
                        for tt in range(4, TB):
                            ti = tb * TB + tt
                            nc.tensor.matmul(ps_o[:], pts_blk[:, tt * H:(tt + 1) * H],
                                             vt_blk[:, tt * KVLR:(tt + 1) * KVLR],
                                             start=(ti == 0), stop=(ti == NT - 1))

                    stot = a_misc.tile([H, 1], F32, tag="stot")
                    nc.vector.tensor_reduce(stot[:], sums[:], mybir.AxisListType.X,
                                            mybir.AluOpType.add)
                    nc.vector.reciprocal(stot[:], stot[:])
                    o_sb = a_misc.tile([H, KVLR], F16, tag="o_sb")
                    nc.scalar.activation(o_sb[:], ps_o[:], AF.Copy, scale=stot[:])
                    if not F_SPLIT2:
                        nc.gpsimd.dma_start(a2a2_in[:, bl, :, :], o_sb[:])
                    elif bl < 2:
                        nc.gpsimd.dma_start(a2a2_in_a[:, bl, :, :], o_sb[:])
                    else:
                        nc.gpsimd.dma_start(a2a2_in_b[:, bl - 2, :, :], o_sb[:])
                    if bl == 0:
                        wo_gate_release(range(NGC[1], NGC[2]), p5wc, o_sb[:1, :1])
                    elif bl == 1:
                        wo_gate_release(range(NGC[2], NGC[3]), p5wc, o_sb[:1, :1])
                        if F_SPLIT2:
                            # first half A2A2 overlaps attention of batches 2-3
                            collective("AllToAll", mybir.AluOpType.bypass, RG,
                                       [a2a2_in_a[:].flatten()], [a2a2_out_a[:].flatten()])
                    elif bl == 2:
                        wo_gate_release(range(NGC[3], NG2), p5wb1, o_sb[:1, :1])
                    elif bl == 3:
                        wo_gate_release(range(NG2, NG3), p5wb2, o_sb[:1, :1])

                if F_SPLIT2:
                    collective("AllToAll", mybir.AluOpType.bypass, RG,
                               [a2a2_in_b[:].flatten()], [a2a2_out_b[:].flatten()])
                else:
                    collective("AllToAll", mybir.AluOpType.bypass, RG,
                               [a2a2_in[:].flatten()], [a2a2_out[:].flatten()])

            # ---------------- Phase 4: uv projection -> o_uv^T tiles ----------------
            with tc.tile_pool(name="p4", bufs=3) as p4, \
                 tc.tile_pool(name="p4ps", bufs=2, space="PSUM") as p4ps, \
                 tc.tile_pool(name="p4psT", bufs=2, space="PSUM") as p4psT:
                NCT = KVLR // 128  # 4
                if F_SPLIT2:
                    # per batch-half: gather [16,512] (contiguous partitions),
                    # transpose, partial uv matmuls, scatter into obT columns
                    # (free-dim strides only -- proven-safe patterns)
                    for half, a2a2_out_h in ((0, a2a2_out_a), (1, a2a2_out_b)):
                        oh_ts = []
                        for h in range(HL):
                            oh_t = p4.tile([16, KVLR], F16, tag=f"oh{half}_{h}",
                                           name=f"oh{half}_{h}", bufs=1)
                            nc.gpsimd.dma_start(oh_t[:], a2a2_out_h[:, :, h, :])
                            oh_ts.append(oh_t)
                        for h in range(HL):
                            oh = oh_ts[h][:]
                            ohps = p4psT.tile([128, NCT * 16], F16, tag="ohps")
                            for ct in range(NCT):
                                nc.tensor.transpose(
                                    ohps[:, ct * 16:(ct + 1) * 16],
                                    oh[:, ct * 128:(ct + 1) * 128],
                                    identh[:16, :16])
                            ohh = p4.tile([128, NCT * 16], F16, tag="ohh")
                            nc.vector.tensor_copy(ohh[:], ohps[:])
                            psuv = p4ps.tile([DV, 16], F32, tag="psuv")
                            for ct in range(NCT):
                                nc.tensor.matmul(
                                    psuv[:], wuv[:, (h * NCT + ct) * DV:(h * NCT + ct + 1) * DV],
                                    ohh[:, ct * 16:(ct + 1) * 16],
                                    start=(ct == 0), stop=(ct == NCT - 1))
                            # scatter into global-batch column order r*4+(2*half+bl)
                            dstv = obT_all[:, h * B:(h + 1) * B].rearrange(
                                "p (r bl) -> p r bl", bl=BL)[:, :, 2 * half:2 * half + 2]
                            srcv = psuv[:].rearrange("p (r bl) -> p r bl", bl=2)
                            if h % 2 == 0:
                                nc.vector.tensor_copy(dstv, srcv)
                            else:
                                nc.scalar.copy(dstv, srcv)
                else:
                    oh_ts = []
                    for h in range(HL):
                        oh_t = p4.tile([B, KVLR], F16, tag=f"oh{h}", name=f"oh{h}", bufs=1)
                        nc.gpsimd.dma_start(oh_t[:], a2a2_out[:, :, h, :])
                        oh_ts.append(oh_t)
                    for h in range(HL):
                        oh = oh_ts[h][:]
                        ohps = p4psT.tile([128, NCT * B], F16, tag="ohps")
                        for ct in range(NCT):
                            nc.tensor.transpose(
                                ohps[:, ct * B:(ct + 1) * B],
                                oh[:, ct * 128:(ct + 1) * 128],
                                identh[:B, :B])
                        ohh = p4.tile([128, NCT * B], F16, tag="ohh")
                        nc.vector.tensor_copy(ohh[:], ohps[:])
                        psuv = p4ps.tile([DV, B], F32, tag="psuv")
                        for ct in range(NCT):
                            nc.tensor.matmul(
                                psuv[:], wuv[:, (h * NCT + ct) * DV:(h * NCT + ct + 1) * DV],
                                ohh[:, ct * B:(ct + 1) * B],
                                start=(ct == 0), stop=(ct == NCT - 1))
                        nc.vector.tensor_copy(obT_all[:, h * B:(h + 1) * B], psuv[:])
            if F_DBG:
                nc.gpsimd.dma_start(dbg_obT, obT_all[:])
            # ---------------- Phase 5: partial out = o_uv @ wo^T (my heads) ----------------
            # col-tiled: each quad q has W5[q]//512 col-groups running
            # concurrently; PSUM tile [128, 512] holds all of them
            with tc.tile_pool(name="p5", bufs=3) as p5, \
                 tc.tile_pool(name="p5cyc", bufs=4) as p5cyc, \
                 tc.tile_pool(name="p5ps", bufs=2, space="PSUM") as p5ps:
                rs_bufs = [(rs_in_a, rs_out_a, 0), (rs_in_b, rs_out_b, 4096),
                           (rs_in_c, rs_out_c, 6144)]
                for q in range(NQUAD):
                    ncg = W5[q] // 512
                    pw = min(ncg, 4) * 32       # psum partition rows used
                    pcw = 512 * ((ncg + 3) // 4)  # psum cols (1 or 2 banks)
                    ps5 = p5ps.tile([128, 1024], F32, tag="ps5")
                    for k in range(NK5):
                        ti = q * NK5 + k
                        if ti in wo_tiles:
                            wt = wo_tiles[ti]
                        else:
                            wt = wo_tile_dma(ti, p5cyc)
                        for n in range(ncg):
                            j, m = n % 4, n // 4
                            nc.tensor.matmul(
                                ps5[32 * j:32 * j + B, m * 512:(m + 1) * 512],
                                obT_all[:, k * B:(k + 1) * B],
                                wt[:, n * 512:(n + 1) * 512],
                                start=(k == 0), stop=(k == NK5 - 1),
                                tile_position=(0, 32 * j))
                    so = p5.tile([128, 1024], F16, tag="so")
                    nc.vector.tensor_copy(so[:pw, :pcw], ps5[:pw, :pcw])
                    dst, rso, off = rs_bufs[q]
                    for n in range(ncg):
                        j, m = n % 4, n // 4
                        nc.gpsimd.dma_start(
                            dst[:, n * 512:(n + 1) * 512],
                            so[32 * j:32 * j + B, m * 512:(m + 1) * 512])
                    collective("ReduceScatter", mybir.AluOpType.add, RG,
                               [dst[:].flatten()], [rso[:].flatten()])
                for q in range(NQUAD):
                    dst, rso, off = rs_bufs[q]
                    nc.gpsimd.dma_start(out_part[:, off:off + W5[q]], rso[:])

    nc.compile()
    return nc


def _get_nc():
    if "nc" not in _CACHE:
        _CACHE["nc"] = _build()
    return _CACHE["nc"]


def _make_in_maps(x, freqs_cos, freqs_sin, kv_cache, pe_cache, wq_a, q_norm_w,
                  wq_b, wkv_a, kv_norm_w, wkv_b, wo):
    f16 = np.float16
    # x^T pre-tiled to the SBUF layout: x_p[p, k*B + b] = x[b, k*128 + p]
    x2 = np.ascontiguousarray(
        np.asarray(x, dtype=np.float32).reshape(B, DIM).T
        .reshape(DIM // 128, 128, B).transpose(1, 0, 2).reshape(128, -1)
        .astype(f16))
    wq_a_t = np.asarray(wq_a, dtype=np.float32).T    # [DIM, QLR] view
    wkv_a_t = np.asarray(wkv_a, dtype=np.float32).T  # [DIM, KVLR+DR] view
    # fold the rmsnorm elementwise weight into wq_b
    wq_b_np = (np.asarray(wq_b, dtype=np.float32)
               * np.asarray(q_norm_w, dtype=np.float32).reshape(1, QLR))
    wkv_b_r = np.asarray(wkv_b, dtype=np.float32).reshape(H, DN + DV, KVLR)
    wo_T = np.asarray(wo, dtype=np.float32).T        # [H*DV, DIM] view
    fc = np.ascontiguousarray(np.broadcast_to(
        np.asarray(freqs_cos, dtype=np.float32).reshape(1, DR // 2), (B, DR // 2)))
    fs = np.ascontiguousarray(np.broadcast_to(
        np.asarray(freqs_sin, dtype=np.float32).reshape(1, DR // 2), (B, DR // 2)))
    knw = np.ascontiguousarray(np.broadcast_to(
        np.asarray(kv_norm_w, dtype=np.float32).reshape(1, KVLR), (B, KVLR)))
    kv_np = np.asarray(kv_cache, dtype=np.float32)
    pe_np = np.asarray(pe_cache, dtype=np.float32)

    in_maps = []
    for r in range(NCORES):
        hs = slice(r * HL, (r + 1) * HL)
        bs = slice(r * BL, (r + 1) * BL)
        # [wq_a^T | wkv_a^T] column slice, pretiled to the SBUF layout:
        # wqkva_p[p, k*PL + e] = wqkva_t[k*128 + p, e]
        wqkva = np.concatenate(
            [wq_a_t[:, r * QL:(r + 1) * QL], wkv_a_t[:, r * KL:(r + 1) * KL]],
            axis=1)   # [DIM, PL]
        wqkva_p = np.ascontiguousarray(
            wqkva.reshape(NKT, 128, PL).transpose(1, 0, 2).reshape(128, -1)
            .astype(f16))
        # wkv_b nope rows in SBUF layout: wn_p[p, h*KVLR + c] = wkv_b[h, p, c]
        wn_pm = np.ascontiguousarray(
            wkv_b_r[hs, :DN, :].transpose(1, 0, 2).reshape(128, -1).astype(f16))
        in_maps.append({
            "x_p": x2,
            "wqkva_p": wqkva_p,
            "wq_b_t": np.ascontiguousarray(
                wq_b_np[r * HL * QKD:(r + 1) * HL * QKD, :].T.astype(f16)),
            "wn_p": wn_pm,
            # uv weights in SBUF layout: [p, (h*4+ct)*DV + dv] = wkv_b[h, DN+dv, ct*128+p]
            "wuv_p": np.ascontiguousarray(
                wkv_b_r[hs, DN:, :].transpose(0, 2, 1)   # [h, c, dv]
                .reshape(HL, KVLR // 128, 128, DV)        # [h, ct, p, dv]
                .transpose(2, 0, 1, 3).reshape(128, -1).astype(f16)),
            "wo_th": np.ascontiguousarray(wo_T[r * KD:(r + 1) * KD, :].astype(f16)),
            "kv_t": np.ascontiguousarray(kv_np[bs].transpose(0, 2, 1).astype(f16)),
            "pe_t": np.ascontiguousarray(pe_np[bs].transpose(0, 2, 1).astype(f16)),
            "kv_norm_w": knw, "fcos": fc, "fsin": fs,
        })
    return in_maps


def _get_runner():
    """Cached jitted SPMD executable (mirrors bass2jax.run_bass_via_pjrt, but
    reuses one jax.jit object so warm calls skip retracing/recompiling)."""
    if "runner" in _CACHE:
        return _CACHE["runner"]
    import jax
    from concourse import bass2jax
    from jax.experimental.shard_map import shard_map
    from jax.sharding import Mesh, PartitionSpec
    import concourse.mybir as mybir_

    nc = _get_nc()
    bass2jax.install_neuronx_cc_hook()
    part_name = nc.partition_id_tensor.name if nc.partition_id_tensor else None
    in_names, out_names, out_avals = [], [], []
    for alloc in nc.m.functions[0].allocations:
        if not isinstance(alloc, mybir_.MemoryLocationSet):
            continue
        name = alloc.memorylocations[0].name
        if alloc.kind == "ExternalInput":
            if name != part_name:
                in_names.append(name)
        elif alloc.kind == "ExternalOutput":
            out_names.append(name)
            out_avals.append(jax.core.ShapedArray(
                tuple(alloc.tensor_shape), mybir_.dt.np(alloc.dtype)))
    n_params = len(in_names)
    all_names = in_names + out_names + ([part_name] if part_name else [])

    def _body(*args):
        operands = list(args)
        if part_name:
            operands.append(bass2jax.partition_id_tensor())
        outs = bass2jax._bass_exec_p.bind(
            *operands, out_avals=tuple(out_avals), in_names=tuple(all_names),
            out_names=tuple(out_names), lowering_input_output_aliases=(),
            sim_require_finite=True, sim_require_nnan=True, nc=nc)
        return tuple(outs)

    devices = jax.devices()[:NCORES]
    mesh = Mesh(np.asarray(devices), ("core",))
    n_outs = len(out_names)
    donate = tuple(range(n_params, n_params + n_outs))
    sharded = jax.jit(
        shard_map(_body, mesh=mesh,
                  in_specs=(PartitionSpec("core"),) * (n_params + n_outs),
                  out_specs=(PartitionSpec("core"),) * n_outs,
                  check_rep=False),
        donate_argnums=donate, keep_unused=True)
    _CACHE["runner"] = (sharded, in_names, out_names, out_avals)
    return _CACHE["runner"]


def _run(in_maps):
    sharded, in_names, out_names, out_avals = _get_runner()
    concat_in = [np.concatenate([in_maps[c][n] for c in range(NCORES)], axis=0)
                 for n in in_names]
    concat_zeros = [np.zeros((NCORES * a.shape[0], *a.shape[1:]), a.dtype)
                    for a in out_avals]
    out_arrs = sharded(*concat_in, *concat_zeros)
    return {n: np.asarray(out_arrs[i]) for i, n in enumerate(out_names)}


def _assemble(parts):
    # parts[r] is [BL, DIM] (fp16) for batches r*BL .. (r+1)*BL-1
    out = np.concatenate(
        [np.asarray(p).astype(np.float32).reshape(BL, DIM) for p in parts], axis=0)
    return np.ascontiguousarray(out.reshape(B, 1, DIM))


def kernel(x, freqs_cos, freqs_sin, kv_cache, pe_cache, wq_a, q_norm_w,
           wq_b, wkv_a, kv_norm_w, wkv_b, wo, start_pos, _trace=False):
    assert int(start_pos) == SPOS, f"kernel compiled for start_pos={SPOS}"
    in_maps = _make_in_maps(x, freqs_cos, freqs_sin, kv_cache, pe_cache, wq_a,
                            q_norm_w, wq_b, wkv_a, kv_norm_w, wkv_b, wo)
    if _trace:
        import tempfile
        tmpdir = tempfile.mkdtemp(prefix="mla_trace_")
        res = run_bass_kernel_spmd(_get_nc(), in_maps,
                                   core_ids=list(range(NCORES)),
                                   trace=True, tmpdir=tmpdir)
        out = _assemble([res.results[r]["out_part"] for r in range(NCORES)])
        print(f"trace tmpdir: {tmpdir}")
        if res.instructions_and_trace is not None:
            print(f"trace path: {res.instructions_and_trace[1]}")
        return out, res
    outs = _run(in_maps)
    return _assemble(outs["out_part"].reshape(NCORES, BL, DIM))
